# revision 21
# baseline (speedup 1.0000x reference)
"""Trainium2 Bass kernel for PoseOptimizerLayer's build_q_matrix.

Math: every entry of the (5,5) Q is a bilinear form in per-point features
  phi(a_i) = [1, x_a, y_a, x_a^2+y_a^2]   (Na x 4)
  psi(b_j) = [1, x_b, y_b, x_b^2+y_b^2]   (Nb x 4)
through the association-weighted moment matrix
  S = phi^T A psi                          (4 x 4 per batch)
and Q_flat(25) = TmatQ^T @ s_flat for a constant TmatQ.

Device plan (per core, 2 of the 16 batches; data-parallel over batch, no
collectives).  The kernel is HBM-bound on streaming A (2 bytes/elem,
16MB/core); everything else is shaped to keep that stream saturated and
the PE warm:

  A quantization -- bf16 + Hilbert sort + error diffusion: the host
    centers A at its distribution mean (a-0.5), PERMUTES the j columns
    into Hilbert-curve order of (x_b, y_b) (the kernel's pair sum is
    j-permutation invariant), and rounds to bf16 with row-wise error
    diffusion: the rounding residual of column j is added to column j+1
    before rounding.  Because neighboring columns in Hilbert order have
    nearly equal psi features, the diffused noise telescopes in every
    moment sum: worst Q-entry rel err 2.4e-3 (vs 3.5e-3 for plain
    centered fp16, 8.3e-2 for plain bf16; gate 2e-2).  bf16 streams
    through the PE at full rate and halves HBM traffic vs fp32.
    The exact rank-1 mean term 0.5*(sum phi)(sum psi) is host-computed
    (corrq) and rides into stage 3 as an extra s-column.

  stage 1: P32 = PhiHL^T A  (32 x Nb) -- bf16 PE matmuls into PSUM.
    The (128 x 32) stationary holds phi split into bf16 hi+lo halves
    (col 16h+4pp+q = phi_pp part h, q-replicated), host-prebuilt.
    A streams on the sync HWDGE queue in large contiguous DMAs with
    contiguous-per-partition layout: partition p holds k consecutive A
    rows (16KB descriptors at k=4).  The first chunks of batch 0 are
    small (k=1,1,2) so real matmuls start early.  Accumulation is split
    over TWO bank sets (i-chunks 0-11 -> banks A, 12-15 -> banks B) so
    group A's stage-2 runs mid-stream without stalling the PE.
  warm-up: ~24 zero matmuls into bank A0 right after the preamble keep
    the PE HAM-unthrottled (2.4 GHz) before the first A chunk lands.
  stage 2: per-bank DVE multiply against host-prebuilt replicated psi
    rows (g_rep, fp32) + scalar-engine activation-accumulate into one
    s-column.  Only group B's 4 columns + stage 3 remain after the
    final A byte.
  stage 3: per batch: two narrow DVE reduces (5+4 cols -- a single
    9-wide X-reduce double-counts its full 8-chunk on HW) + add ->
    s_tot(32,1), one tiny matmul q = tmatq2^T @ s_tot into a corner of
    bank A0, DVE copy out, output DMA on the sync queue.
"""

import os
import numpy as np

BATCH, NA, NB = 16, 2048, 2048
N_CORES = 8
BL = BATCH // N_CORES  # batches per core
P = 128
IC = NA // P   # 128-row i-blocks per batch
NJ = 512       # moving-operand width = one PSUM bank
JC = NB // NJ  # j-chunks

# rows-per-partition (k) per A-stream DMA; sum(k) == IC per batch.
KPAT0 = (1, 1, 2, 4, 4, 4)
KPAT1 = (4, 4, 4, 4)
GROUP_IC = 12  # stage-2 accumulation-group boundary (i-blocks)

WARM_MMS = int(os.environ.get("KERNEL_WARM", "24"))
A_BUFS = int(os.environ.get("KERNEL_A_BUFS", "6"))
USE_ACT = os.environ.get("KERNEL_ACT", "1") == "1"
OUT_SYNC = os.environ.get("KERNEL_OUT", "sync") == "sync"
USE_BF16 = os.environ.get("KERNEL_BF16", "1") == "1"

LAST_RESULTS = None  # test harness can inspect exec_time_ns etc.


def _tmatq() -> np.ndarray:
    """(16, 25): row 4pp+q = coeff of S[pp][q] in Q_flat[k]."""
    T = np.zeros((16, 25), np.float32)

    def s(p, q):
        return 4 * p + q

    entries = [
        (s(0, 3), 0, 1.0),                      # q00 = S03
        (s(0, 1), 1, -1.0), (s(0, 1), 5, -1.0),   # q01 = -S01
        (s(0, 2), 2, -1.0), (s(0, 2), 10, -1.0),  # q02 = -S02
        (s(1, 1), 3, -1.0), (s(2, 2), 3, -1.0),   # q03 = -(S11+S22)
        (s(1, 1), 15, -1.0), (s(2, 2), 15, -1.0),
        (s(2, 1), 4, 1.0), (s(1, 2), 4, -1.0),    # q04 = S21-S12
        (s(2, 1), 20, 1.0), (s(1, 2), 20, -1.0),
        (s(0, 0), 6, 1.0), (s(0, 0), 12, 1.0),    # w = S00
        (s(1, 0), 8, 1.0), (s(1, 0), 16, 1.0),    # q13 = q24 = S10
        (s(1, 0), 14, 1.0), (s(1, 0), 22, 1.0),
        (s(2, 0), 9, -1.0), (s(2, 0), 21, -1.0),  # q14 = -S20
        (s(2, 0), 13, 1.0), (s(2, 0), 17, 1.0),   # q23 = S20
        (s(3, 0), 18, 1.0), (s(3, 0), 24, 1.0),   # q33 = S30
    ]
    for si, qi, v in entries:
        T[si, qi] += v
    return T


def _block_rows(kpat) -> np.ndarray:
    """rows[ic, p] = A row held at partition p, stationary block ic."""
    rows = np.zeros((IC, P), np.int64)
    r0 = 0
    ic = 0
    for k in kpat:
        for s in range(k):
            rows[ic + s] = r0 + np.arange(P) * k + s
        ic += k
        r0 += P * k
    return rows


def _hilbert_index(xq: np.ndarray, yq: np.ndarray, order: int = 16) -> np.ndarray:
    idx = np.zeros(len(xq), dtype=np.uint64)
    x = xq.astype(np.int64).copy()
    y = yq.astype(np.int64).copy()
    s = 1 << (order - 1)
    while s > 0:
        rx = ((x & s) > 0).astype(np.int64)
        ry = ((y & s) > 0).astype(np.int64)
        idx += (s * s * ((3 * rx) ^ ry)).astype(np.uint64)
        m0 = ry == 0
        m1 = m0 & (rx == 1)
        x[m1] = s - 1 - x[m1]
        y[m1] = s - 1 - y[m1]
        tx = x[m0].copy()
        x[m0] = y[m0]
        y[m0] = tx
        s >>= 1
    return idx


NCOL = 2 * JC + 1  # s-columns per batch: groupA(4) + groupB(4) + corr(1)

_BUILT = None


def _build():
    global _BUILT
    if _BUILT is not None:
        return _BUILT
    import concourse.bass as bass
    import concourse.mybir as mybir
    import concourse.tile as tile
    from concourse import bacc

    f32 = mybir.dt.float32
    dt16 = mybir.dt.bfloat16 if USE_BF16 else mybir.dt.float16
    alu = mybir.AluOpType

    nc = bacc.Bacc("TRN2", target_bir_lowering=False, debug=False)
    A = nc.dram_tensor("associations", [BL, NA, NB], dt16, kind="ExternalInput")
    ph = nc.dram_tensor("phiq", [P, BL * IC * 32], dt16, kind="ExternalInput")
    gr = nc.dram_tensor("grepq", [BL, 32, NB], f32, kind="ExternalInput")
    cr = nc.dram_tensor("corrq", [32, BL], f32, kind="ExternalInput")
    tm = nc.dram_tensor("tmatq", [32, 25], f32, kind="ExternalInput")
    qo = nc.dram_tensor("q_out", [BL, 5, 5], f32, kind="ExternalOutput")

    with tile.TileContext(nc) as tc:
        with (
            tc.tile_pool(name="const", bufs=1) as cpool,
            tc.tile_pool(name="asml", bufs=2) as apool_s,
            tc.tile_pool(name="abig", bufs=A_BUFS) as apool,
            tc.tile_pool(name="small", bufs=1) as spool,
            tc.tile_pool(name="psp", bufs=1, space=bass.MemorySpace.PSUM) as psp,
        ):
            # PSUM: group A banks double as the warm-up target (A0) and the
            # stage-3 matmul output corner (A0); group B gets its own 4.
            banksA = [
                psp.tile([32, NJ], f32, tag=f"pa{jc}", name=f"pa{jc}")
                for jc in range(JC)
            ]
            banksB = [
                psp.tile([32, NJ], f32, tag=f"pb{jc}", name=f"pb{jc}")
                for jc in range(JC)
            ]

            # ---- PE warm-up: zero matmuls, emitted first so they head the
            # PE queue and un-throttle HAM before the first A chunk lands.
            warm_mv = cpool.tile([P, NJ], dt16, tag="warm")
            nc.vector.memset(warm_mv[:], 0.0)
            for w in range(WARM_MMS):
                nc.tensor.matmul(
                    banksA[0][:], warm_mv[:, 0:32], warm_mv[:],
                    start=True, stop=True,
                )

            # ---- constants / features (gpsimd + scalar queues, off the
            # sync queue that carries the A stream)
            tmat_sb = cpool.tile([32, 25], f32, tag="tmat")
            nc.gpsimd.dma_start(tmat_sb[:], tm[:])
            s_all = cpool.tile([32, BL * NCOL], f32, tag="sall")
            for b in range(BL):
                nc.gpsimd.dma_start(
                    s_all[:, b * NCOL + 2 * JC : b * NCOL + 2 * JC + 1],
                    cr[:, b : b + 1],
                )
            f_all = cpool.tile([P, BL * IC * 32], dt16, tag="fall")
            nc.scalar.dma_start(f_all[:], ph[:])
            greps = []
            for b in range(BL):
                g_sb = cpool.tile([32, NB], f32, tag=f"g{b}")
                nc.scalar.dma_start(g_sb[:], gr[b])
                greps.append(g_sb)

            w32 = spool.tile([32, NJ], f32, tag="w32")
            wdump = spool.tile([32, NJ], f32, tag="wdump")

            def stage2(b, grp, jc):
                col = b * NCOL + grp * JC + jc
                bank = (banksA if grp == 0 else banksB)[jc]
                nc.vector.tensor_mul(
                    w32[:], bank[:], greps[b][:, jc * NJ : (jc + 1) * NJ]
                )
                if USE_ACT:
                    nc.scalar.activation(
                        wdump[:], w32[:],
                        mybir.ActivationFunctionType.Copy,
                        accum_out=s_all[:, col : col + 1],
                    )
                else:
                    nc.vector.tensor_reduce(
                        s_all[:, col : col + 1], w32[:],
                        mybir.AxisListType.X, alu.add,
                    )

            for b in range(BL):
                kpat = KPAT0 if b == 0 else KPAT1
                ic = 0
                r0 = 0
                for ci, k in enumerate(kpat):
                    if k < 4:
                        a_t = apool_s.tile([P, k * NB], dt16, tag=f"a{k}",
                                           name=f"a{b}_{ci}")
                    else:
                        a_t = apool.tile([P, k * NB], dt16, tag="a4",
                                         name=f"a{b}_{ci}")
                    aview = a_t[:].rearrange("p (s j) -> p s j", j=NB)
                    asrc = A[b, r0 : r0 + P * k, :].rearrange(
                        "(p s) j -> p s j", s=k
                    )
                    if ci == len(kpat) - 1:
                        # split the final chunk by j-halves so banks 0/1's
                        # last matmuls start before the full completion
                        nc.sync.dma_start(aview[:, :, 0 : NB // 2],
                                          asrc[:, :, 0 : NB // 2])
                        nc.sync.dma_start(aview[:, :, NB // 2 : NB],
                                          asrc[:, :, NB // 2 : NB])
                    else:
                        nc.sync.dma_start(aview, asrc)
                    for s in range(k):
                        icb = ic + s
                        banks = banksA if icb < GROUP_IC else banksB
                        lhs = f_all[:, (b * IC + icb) * 32 : (b * IC + icb + 1) * 32]
                        for jc in range(JC):
                            nc.tensor.matmul(
                                banks[jc][:],
                                lhs,
                                a_t[:, s * NB + jc * NJ : s * NB + (jc + 1) * NJ],
                                start=(icb == 0 or icb == GROUP_IC),
                                stop=(icb == GROUP_IC - 1 or icb == IC - 1),
                            )
                    ic += k
                    r0 += P * k
                    if ic == GROUP_IC:
                        # group A closed: reduce it while group B streams
                        for jc in range(JC):
                            stage2(b, 0, jc)
                for jc in range(JC):
                    stage2(b, 1, jc)

            # ---- stage 3 per batch: narrow reduces (a 9-wide X-reduce
            # double-counts its full 8-chunk on HW) + tiny matmul into a
            # corner of bank A0 + copy out on the sync queue.
            for b in range(BL):
                s_tot = spool.tile([32, 1], f32, tag=f"stot{b}")
                s_t2 = spool.tile([32, 1], f32, tag=f"st2{b}")
                nc.vector.tensor_reduce(
                    s_tot[:], s_all[:, b * NCOL : b * NCOL + 5],
                    mybir.AxisListType.X, alu.add,
                )
                nc.vector.tensor_reduce(
                    s_t2[:], s_all[:, b * NCOL + 5 : (b + 1) * NCOL],
                    mybir.AxisListType.X, alu.add,
                )
                nc.vector.tensor_add(s_tot[:], s_tot[:], s_t2[:])
                q_ps = psp.tile([32, NJ], f32, tag="pa0", name=f"q{b}")
                nc.tensor.matmul(
                    q_ps[0:25, 0:1], tmat_sb[:], s_tot[:], start=True, stop=True,
                )
                q_sb = spool.tile([25, 1], f32, tag=f"qsb{b}")
                nc.vector.tensor_copy(q_sb[:], q_ps[0:25, 0:1])
                eng = nc.sync if OUT_SYNC else nc.gpsimd
                eng.dma_start(qo[b].rearrange("a b -> (a b)"), q_sb[:, 0])

    nc.compile()
    _BUILT = nc
    return nc


def prep_in_maps(associations, pt_in_a, pt_in_b):
    import ml_dtypes

    bf16 = ml_dtypes.bfloat16
    np16 = bf16 if USE_BF16 else np.float16
    tq = _tmatq()
    tmatq = np.concatenate([tq, tq], axis=0)  # (32, 25): folds hi+lo halves

    pt_in_b = np.asarray(pt_in_b, dtype=np.float32)
    xb, yb = pt_in_b[..., 0], pt_in_b[..., 1]
    # Hilbert-curve order of the b-points, per batch: makes psi features
    # smooth along j so the diffused bf16 noise telescopes.
    def qz(v):
        v = (v - v.min()) / (v.max() - v.min() + 1e-12)
        return np.clip((v * 65535).astype(np.int64), 0, 65535)
    perms = np.empty((BATCH, NB), np.int64)
    for g in range(BATCH):
        perms[g] = np.argsort(
            _hilbert_index(qz(xb[g]), qz(yb[g])), kind="stable"
        )
    xbp = np.take_along_axis(xb, perms, axis=1)
    ybp = np.take_along_axis(yb, perms, axis=1)

    associations = np.ascontiguousarray(associations, dtype=np.float32)
    Ac = np.empty_like(associations)
    for g in range(BATCH):
        Ac[g] = associations[g][:, perms[g]]
    Ac -= np.float32(0.5)  # center at the distribution mean
    Aq = np.empty(Ac.shape, np16)
    if USE_BF16:
        # row-wise error diffusion along the Hilbert-ordered j axis
        e = np.zeros(Ac.shape[:2], np.float32)
        for j in range(NB):
            v = Ac[:, :, j] + e
            qv = v.astype(bf16)
            e = v - qv.astype(np.float32)
            Aq[:, :, j] = qv
    else:
        Aq[...] = Ac.astype(np16)

    pt_in_a = np.asarray(pt_in_a, dtype=np.float32)
    xa, ya = pt_in_a[..., 0], pt_in_a[..., 1]
    phi = np.stack([np.ones_like(xa), xa, ya, xa * xa + ya * ya], axis=-1)
    phi_hi = phi.astype(np16).astype(np.float32)
    phi_lo = (phi - phi_hi).astype(np16).astype(np.float32)

    # phi stationary, permuted to match the contiguous-per-partition A
    # layout: block ic holds phi[rows[ic, p]] at partition p, with col
    # layout ic*32 + 16h + 4pp + q (q replicated 4x).
    rows0 = _block_rows(KPAT0)
    rows1 = _block_rows(KPAT1)
    phiq = np.zeros((BATCH, P, IC, 2, 4, 4), np16)
    for g in range(BATCH):
        rows = rows0 if (g % BL) == 0 else rows1
        for h, php in ((0, phi_hi), (1, phi_lo)):
            blk = php[g][rows]            # (IC, P, 4)
            blk = blk.transpose(1, 0, 2)  # (P, IC, 4)
            phiq[g, :, :, h, :, :] = blk.astype(np16)[..., None]
    phiq = phiq.reshape(BATCH, P, IC * 32)

    psi = np.stack([np.ones_like(xbp), xbp, ybp, xbp * xbp + ybp * ybp], axis=1)
    # (B, 4, Nb) -> replicated (B, 32, Nb): row 16h+4pp+q = psi_q
    grepq = np.ascontiguousarray(
        np.broadcast_to(psi[:, None, None, :, :], (BATCH, 2, 4, 4, NB))
        .reshape(BATCH, 32, NB)
    )

    # mean-correction column: 0.5 * (sum_i phi^h_pp) * (sum_j psi_q)
    sphi = np.stack([phi_hi.sum(axis=1), phi_lo.sum(axis=1)], axis=1)  # (B,2,4)
    spsi = psi.sum(axis=2)  # (B, 4)
    corrq = 0.5 * np.einsum('bhp,bq->bhpq', sphi.astype(np.float64),
                            spsi.astype(np.float64))
    corrq = corrq.reshape(BATCH, 32).astype(np.float32)

    in_maps = []
    for c in range(N_CORES):
        sl = slice(c * BL, (c + 1) * BL)
        ph_core = np.ascontiguousarray(
            phiq[sl].transpose(1, 0, 2).reshape(P, BL * IC * 32)
        )
        in_maps.append(
            {
                "associations": np.ascontiguousarray(Aq[sl]),
                "phiq": ph_core,
                "grepq": np.ascontiguousarray(grepq[sl]),
                "corrq": np.ascontiguousarray(corrq[sl].T),
                "tmatq": tmatq,
            }
        )
    return in_maps


def kernel(associations: np.ndarray, pt_in_a: np.ndarray, pt_in_b: np.ndarray
           ) -> np.ndarray:
    global LAST_RESULTS
    from concourse.bass_utils import run_bass_kernel_spmd

    nc = _build()
    in_maps = prep_in_maps(associations, pt_in_a, pt_in_b)
    res = run_bass_kernel_spmd(nc, in_maps, list(range(N_CORES)))
    LAST_RESULTS = res
    out = np.concatenate([res.results[c]["q_out"] for c in range(N_CORES)], axis=0)
    return out.astype(np.float32, copy=False)


# revision 22
# speedup vs baseline: 1.1398x; 1.1398x over previous
"""Trainium2 Bass kernel for PoseOptimizerLayer's build_q_matrix.

Math: every entry of the (5,5) Q is a bilinear form in per-point features
  phi(a_i) = [1, x_a, y_a, x_a^2+y_a^2]   (Na x 4)
  psi(b_j) = [1, x_b, y_b, x_b^2+y_b^2]   (Nb x 4)
through the association-weighted moment matrix
  S = phi^T A psi                          (4 x 4 per batch)
and Q_flat(25) = TmatQ^T @ s_flat for a constant TmatQ.

Device plan (per core, 2 of the 16 batches; data-parallel over batch, no
collectives).  The kernel is HBM-bound on streaming A (2 bytes/elem,
16MB/core); everything else is shaped to keep that stream saturated and
the PE warm:

  A quantization -- bf16 + Hilbert sort + error diffusion: the host
    centers A at its distribution mean (a-0.5), PERMUTES the j columns
    into Hilbert-curve order of (x_b, y_b) (the kernel's pair sum is
    j-permutation invariant), and rounds to bf16 with row-wise error
    diffusion: the rounding residual of column j is added to column j+1
    before rounding.  Because neighboring columns in Hilbert order have
    nearly equal psi features, the diffused noise telescopes in every
    moment sum: worst Q-entry rel err 2.4e-3 (vs 3.5e-3 for plain
    centered fp16, 8.3e-2 for plain bf16; gate 2e-2).  bf16 streams
    through the PE at full rate and halves HBM traffic vs fp32.
    The exact rank-1 mean term 0.5*(sum phi)(sum psi) is host-computed
    (corrq) and rides into stage 3 as an extra s-column.

  stage 1: P32 = PhiHL^T A  (32 x Nb) -- bf16 PE matmuls into PSUM.
    The (128 x 32) stationary holds phi split into bf16 hi+lo halves
    (col 16h+4pp+q = phi_pp part h, q-replicated), host-prebuilt.
    A streams on the sync HWDGE queue in large contiguous DMAs with
    contiguous-per-partition layout: partition p holds k consecutive A
    rows (16KB descriptors at k=4).  The first chunks of batch 0 are
    small (k=1,1,2) so real matmuls start early.  Accumulation is split
    over TWO bank sets (i-chunks 0-11 -> banks A, 12-15 -> banks B) so
    group A's stage-2 runs mid-stream without stalling the PE.
  warm-up: ~24 zero matmuls into bank A0 right after the preamble keep
    the PE HAM-unthrottled (2.4 GHz) before the first A chunk lands.
  stage 2: per-bank DVE multiply against host-prebuilt replicated psi
    rows (g_rep, fp32) + scalar-engine activation-accumulate into one
    s-column.  Only group B's 4 columns + stage 3 remain after the
    final A byte.
  stage 3: per batch: two narrow DVE reduces (5+4 cols -- a single
    9-wide X-reduce double-counts its full 8-chunk on HW) + add ->
    s_tot(32,1), one tiny matmul q = tmatq2^T @ s_tot into a corner of
    bank A0, DVE copy out, output DMA on the sync queue.
"""

import os
import numpy as np

BATCH, NA, NB = 16, 2048, 2048
N_CORES = 8
BL = BATCH // N_CORES  # batches per core
P = 128
IC = NA // P   # 128-row i-blocks per batch
NJ = 512       # moving-operand width = one PSUM bank
JC = NB // NJ  # j-chunks

# rows-per-partition (k) per A-stream DMA; sum(k) == IC per batch.
KPAT0 = (1, 1, 2, 4, 4, 4)
KPAT1 = (4, 4, 4, 4)
GROUP_IC = 12  # stage-2 accumulation-group boundary (i-blocks)

WARM_MMS = int(os.environ.get("KERNEL_WARM", "24"))
A_BUFS = int(os.environ.get("KERNEL_A_BUFS", "6"))
USE_ACT = os.environ.get("KERNEL_ACT", "1") == "1"
OUT_SYNC = os.environ.get("KERNEL_OUT", "sync") == "sync"
USE_BF16 = os.environ.get("KERNEL_BF16", "1") == "1"

LAST_RESULTS = None  # test harness can inspect exec_time_ns etc.


def _tmatq() -> np.ndarray:
    """(16, 25): row 4pp+q = coeff of S[pp][q] in Q_flat[k]."""
    T = np.zeros((16, 25), np.float32)

    def s(p, q):
        return 4 * p + q

    entries = [
        (s(0, 3), 0, 1.0),                      # q00 = S03
        (s(0, 1), 1, -1.0), (s(0, 1), 5, -1.0),   # q01 = -S01
        (s(0, 2), 2, -1.0), (s(0, 2), 10, -1.0),  # q02 = -S02
        (s(1, 1), 3, -1.0), (s(2, 2), 3, -1.0),   # q03 = -(S11+S22)
        (s(1, 1), 15, -1.0), (s(2, 2), 15, -1.0),
        (s(2, 1), 4, 1.0), (s(1, 2), 4, -1.0),    # q04 = S21-S12
        (s(2, 1), 20, 1.0), (s(1, 2), 20, -1.0),
        (s(0, 0), 6, 1.0), (s(0, 0), 12, 1.0),    # w = S00
        (s(1, 0), 8, 1.0), (s(1, 0), 16, 1.0),    # q13 = q24 = S10
        (s(1, 0), 14, 1.0), (s(1, 0), 22, 1.0),
        (s(2, 0), 9, -1.0), (s(2, 0), 21, -1.0),  # q14 = -S20
        (s(2, 0), 13, 1.0), (s(2, 0), 17, 1.0),   # q23 = S20
        (s(3, 0), 18, 1.0), (s(3, 0), 24, 1.0),   # q33 = S30
    ]
    for si, qi, v in entries:
        T[si, qi] += v
    return T


def _block_rows(kpat) -> np.ndarray:
    """rows[ic, p] = A row held at partition p, stationary block ic."""
    rows = np.zeros((IC, P), np.int64)
    r0 = 0
    ic = 0
    for k in kpat:
        for s in range(k):
            rows[ic + s] = r0 + np.arange(P) * k + s
        ic += k
        r0 += P * k
    return rows


def _hilbert_index(xq: np.ndarray, yq: np.ndarray, order: int = 16) -> np.ndarray:
    idx = np.zeros(len(xq), dtype=np.uint64)
    x = xq.astype(np.int64).copy()
    y = yq.astype(np.int64).copy()
    s = 1 << (order - 1)
    while s > 0:
        rx = ((x & s) > 0).astype(np.int64)
        ry = ((y & s) > 0).astype(np.int64)
        idx += (s * s * ((3 * rx) ^ ry)).astype(np.uint64)
        m0 = ry == 0
        m1 = m0 & (rx == 1)
        x[m1] = s - 1 - x[m1]
        y[m1] = s - 1 - y[m1]
        tx = x[m0].copy()
        x[m0] = y[m0]
        y[m0] = tx
        s >>= 1
    return idx


NCOL = 3  # s-columns per batch: groupA, groupB, corr

_BUILT = None


def _build():
    global _BUILT
    if _BUILT is not None:
        return _BUILT
    import concourse.bass as bass
    import concourse.mybir as mybir
    import concourse.tile as tile
    from concourse import bacc

    f32 = mybir.dt.float32
    dt16 = mybir.dt.bfloat16 if USE_BF16 else mybir.dt.float16
    alu = mybir.AluOpType

    nc = bacc.Bacc("TRN2", target_bir_lowering=False, debug=False)
    A = nc.dram_tensor("associations", [BL, NA, NB], dt16, kind="ExternalInput")
    ph = nc.dram_tensor("phiq", [P, BL * IC * 32], dt16, kind="ExternalInput")
    gr = nc.dram_tensor("grepq", [BL, P, NJ], f32, kind="ExternalInput")
    cr = nc.dram_tensor("corrq", [P, BL], f32, kind="ExternalInput")
    tm = nc.dram_tensor("tmatq", [P, 25], f32, kind="ExternalInput")
    qo = nc.dram_tensor("q_out", [BL, 5, 5], f32, kind="ExternalOutput")

    with tile.TileContext(nc) as tc:
        with (
            tc.tile_pool(name="const", bufs=1) as cpool,
            tc.tile_pool(name="asml", bufs=2) as apool_s,
            tc.tile_pool(name="abig", bufs=A_BUFS) as apool,
            tc.tile_pool(name="small", bufs=1) as spool,
            tc.tile_pool(name="psp", bufs=1, space=bass.MemorySpace.PSUM) as psp,
        ):
            # PSUM: ONE [128, NJ] bank per accumulation group -- the 4
            # j-chunks live in the 4 col-groups of the PE array
            # (tile_position), so stage 2 is a single [128, NJ] mul+act
            # per group and stage 3 contracts over 128 partitions.
            bankA = psp.tile([P, NJ], f32, tag="pa", name="pa")
            bankB = psp.tile([P, NJ], f32, tag="pb", name="pb")

            # ---- PE warm-up: zero matmuls, emitted first so they head the
            # PE queue and un-throttle HAM before the first A chunk lands.
            warm_mv = cpool.tile([P, NJ], dt16, tag="warm")
            nc.vector.memset(warm_mv[:], 0.0)
            for w in range(WARM_MMS):
                nc.tensor.matmul(
                    bankA[0:32, :], warm_mv[:, 0:32], warm_mv[:],
                    start=True, stop=True, tile_position=(0, 0),
                )

            # ---- constants / features (gpsimd + scalar queues, off the
            # sync queue that carries the A stream)
            tmat_sb = cpool.tile([P, 25], f32, tag="tmat")
            nc.gpsimd.dma_start(tmat_sb[:], tm[:])
            s_all = cpool.tile([P, BL * NCOL], f32, tag="sall")
            for b in range(BL):
                nc.gpsimd.dma_start(
                    s_all[:, b * NCOL + 2 : b * NCOL + 3],
                    cr[:, b : b + 1],
                )
            f_all = cpool.tile([P, BL * IC * 32], dt16, tag="fall")
            nc.scalar.dma_start(f_all[:], ph[:])
            greps = []
            for b in range(BL):
                g_sb = cpool.tile([P, NJ], f32, tag=f"g{b}")
                nc.scalar.dma_start(g_sb[:], gr[b])
                greps.append(g_sb)

            w128 = [spool.tile([P, NJ], f32, tag=f"w{i}", name=f"w{i}")
                    for i in range(2)]
            wdump = spool.tile([P, NJ], f32, tag="wdump")

            def stage2(b, grp):
                col = b * NCOL + grp
                bank = bankA if grp == 0 else bankB
                w = w128[(b * 2 + grp) % 2]
                nc.vector.tensor_mul(w[:], bank[:], greps[b][:])
                if USE_ACT:
                    nc.scalar.activation(
                        wdump[:], w[:],
                        mybir.ActivationFunctionType.Copy,
                        accum_out=s_all[:, col : col + 1],
                    )
                else:
                    nc.vector.tensor_reduce(
                        s_all[:, col : col + 1], w[:],
                        mybir.AxisListType.X, alu.add,
                    )

            for b in range(BL):
                kpat = KPAT0 if b == 0 else KPAT1
                ic = 0
                r0 = 0
                for ci, k in enumerate(kpat):
                    if k < 4:
                        a_t = apool_s.tile([P, k * NB], dt16, tag=f"a{k}",
                                           name=f"a{b}_{ci}")
                    else:
                        a_t = apool.tile([P, k * NB], dt16, tag="a4",
                                         name=f"a{b}_{ci}")
                    aview = a_t[:].rearrange("p (s j) -> p s j", j=NB)
                    asrc = A[b, r0 : r0 + P * k, :].rearrange(
                        "(p s) j -> p s j", s=k
                    )
                    if ci == len(kpat) - 1:
                        # split the final chunk by j-halves so banks 0/1's
                        # last matmuls start before the full completion
                        nc.sync.dma_start(aview[:, :, 0 : NB // 2],
                                          asrc[:, :, 0 : NB // 2])
                        nc.sync.dma_start(aview[:, :, NB // 2 : NB],
                                          asrc[:, :, NB // 2 : NB])
                    else:
                        nc.sync.dma_start(aview, asrc)
                    for s in range(k):
                        icb = ic + s
                        bank = bankA if icb < GROUP_IC else bankB
                        lhs = f_all[:, (b * IC + icb) * 32 : (b * IC + icb + 1) * 32]
                        for jc in range(JC):
                            nc.tensor.matmul(
                                bank[32 * jc : 32 * (jc + 1), :],
                                lhs,
                                a_t[:, s * NB + jc * NJ : s * NB + (jc + 1) * NJ],
                                start=(icb == 0 or icb == GROUP_IC),
                                stop=(icb == GROUP_IC - 1 or icb == IC - 1),
                                tile_position=(0, 32 * jc),
                            )
                    ic += k
                    r0 += P * k
                    if ic == GROUP_IC:
                        # group A closed: reduce it while group B streams
                        stage2(b, 0)
                stage2(b, 1)

            # ---- stage 3 per batch: 3-wide reduce (safe: <=8) ->
            # s_tot(128,1); the 128-deep contraction of tmat128 sums the
            # 4 col-groups for free; output lands in a corner of bank A.
            for b in range(BL):
                s_tot = spool.tile([P, 1], f32, tag=f"stot{b}")
                nc.vector.tensor_reduce(
                    s_tot[:], s_all[:, b * NCOL : (b + 1) * NCOL],
                    mybir.AxisListType.X, alu.add,
                )
                q_ps = psp.tile([P, NJ], f32, tag="pa", name=f"q{b}")
                nc.tensor.matmul(
                    q_ps[0:25, 0:1], tmat_sb[:], s_tot[:], start=True, stop=True,
                )
                q_sb = spool.tile([25, 1], f32, tag=f"qsb{b}")
                nc.vector.tensor_copy(q_sb[:], q_ps[0:25, 0:1])
                eng = nc.sync if OUT_SYNC else nc.gpsimd
                eng.dma_start(qo[b].rearrange("a b -> (a b)"), q_sb[:, 0])

    nc.compile()
    _BUILT = nc
    return nc


def prep_in_maps(associations, pt_in_a, pt_in_b):
    import ml_dtypes

    bf16 = ml_dtypes.bfloat16
    np16 = bf16 if USE_BF16 else np.float16
    tq = _tmatq()
    tmatq = np.concatenate([tq, tq], axis=0)  # (32, 25): folds hi+lo halves
    tmatq = np.concatenate([tmatq] * JC, axis=0)  # (128, 25): sums col-groups

    pt_in_b = np.asarray(pt_in_b, dtype=np.float32)
    xb, yb = pt_in_b[..., 0], pt_in_b[..., 1]
    # Hilbert-curve order of the b-points, per batch: makes psi features
    # smooth along j so the diffused bf16 noise telescopes.
    def qz(v):
        v = (v - v.min()) / (v.max() - v.min() + 1e-12)
        return np.clip((v * 65535).astype(np.int64), 0, 65535)
    perms = np.empty((BATCH, NB), np.int64)
    for g in range(BATCH):
        perms[g] = np.argsort(
            _hilbert_index(qz(xb[g]), qz(yb[g])), kind="stable"
        )
    xbp = np.take_along_axis(xb, perms, axis=1)
    ybp = np.take_along_axis(yb, perms, axis=1)

    associations = np.ascontiguousarray(associations, dtype=np.float32)
    Ac = np.empty_like(associations)
    for g in range(BATCH):
        Ac[g] = associations[g][:, perms[g]]
    Ac -= np.float32(0.5)  # center at the distribution mean
    Aq = np.empty(Ac.shape, np16)
    if USE_BF16:
        # row-wise error diffusion along the Hilbert-ordered j axis
        e = np.zeros(Ac.shape[:2], np.float32)
        for j in range(NB):
            v = Ac[:, :, j] + e
            qv = v.astype(bf16)
            e = v - qv.astype(np.float32)
            Aq[:, :, j] = qv
    else:
        Aq[...] = Ac.astype(np16)

    pt_in_a = np.asarray(pt_in_a, dtype=np.float32)
    xa, ya = pt_in_a[..., 0], pt_in_a[..., 1]
    phi = np.stack([np.ones_like(xa), xa, ya, xa * xa + ya * ya], axis=-1)
    phi_hi = phi.astype(np16).astype(np.float32)
    phi_lo = (phi - phi_hi).astype(np16).astype(np.float32)

    # phi stationary, permuted to match the contiguous-per-partition A
    # layout: block ic holds phi[rows[ic, p]] at partition p, with col
    # layout ic*32 + 16h + 4pp + q (q replicated 4x).
    rows0 = _block_rows(KPAT0)
    rows1 = _block_rows(KPAT1)
    phiq = np.zeros((BATCH, P, IC, 2, 4, 4), np16)
    for g in range(BATCH):
        rows = rows0 if (g % BL) == 0 else rows1
        for h, php in ((0, phi_hi), (1, phi_lo)):
            blk = php[g][rows]            # (IC, P, 4)
            blk = blk.transpose(1, 0, 2)  # (P, IC, 4)
            phiq[g, :, :, h, :, :] = blk.astype(np16)[..., None]
    phiq = phiq.reshape(BATCH, P, IC * 32)

    psi = np.stack([np.ones_like(xbp), xbp, ybp, xbp * xbp + ybp * ybp], axis=1)
    # (B, 4, Nb) -> replicated rows 16h+4pp+q = psi_q, then fold the 4
    # j-chunks onto the partition axis: grep128[32*jc + r, j'] =
    # psi_q[jc*NJ + j']
    grepq = (
        np.broadcast_to(psi[:, None, None, :, :], (BATCH, 2, 4, 4, NB))
        .reshape(BATCH, 32, NB)
    )
    grepq = np.ascontiguousarray(
        grepq.reshape(BATCH, 32, JC, NJ).transpose(0, 2, 1, 3)
        .reshape(BATCH, P, NJ)
    )

    # mean-correction column: 0.5 * (sum_i phi^h_pp) * (sum_j psi_q),
    # placed in col-group 0 (rows 0-31) only
    sphi = np.stack([phi_hi.sum(axis=1), phi_lo.sum(axis=1)], axis=1)  # (B,2,4)
    spsi = psi.sum(axis=2)  # (B, 4)
    corr32 = 0.5 * np.einsum('bhp,bq->bhpq', sphi.astype(np.float64),
                             spsi.astype(np.float64)).reshape(BATCH, 32)
    corrq = np.zeros((BATCH, P), np.float32)
    corrq[:, 0:32] = corr32.astype(np.float32)

    in_maps = []
    for c in range(N_CORES):
        sl = slice(c * BL, (c + 1) * BL)
        ph_core = np.ascontiguousarray(
            phiq[sl].transpose(1, 0, 2).reshape(P, BL * IC * 32)
        )
        in_maps.append(
            {
                "associations": np.ascontiguousarray(Aq[sl]),
                "phiq": ph_core,
                "grepq": np.ascontiguousarray(grepq[sl]),
                "corrq": np.ascontiguousarray(corrq[sl].T),
                "tmatq": tmatq,
            }
        )
    return in_maps


def kernel(associations: np.ndarray, pt_in_a: np.ndarray, pt_in_b: np.ndarray
           ) -> np.ndarray:
    global LAST_RESULTS
    from concourse.bass_utils import run_bass_kernel_spmd

    nc = _build()
    in_maps = prep_in_maps(associations, pt_in_a, pt_in_b)
    res = run_bass_kernel_spmd(nc, in_maps, list(range(N_CORES)))
    LAST_RESULTS = res
    out = np.concatenate([res.results[c]["q_out"] for c in range(N_CORES)], axis=0)
    return out.astype(np.float32, copy=False)
